# revision 24
# baseline (speedup 1.0000x reference)
"""Trainium2 Bass kernel for the correlation-softargmax flow module.

Math (per batch b, query pixel q=(y,x)):
  c1 = l2norm_C(feature1), warp = l2norm_C(feature2)
  s[l,q] = <3x3 patch of warp at l, 3x3 patch of c1 at q>    (D = 32*9 = 288)
  p = softmax_l(10*s);  flow = (E_p[ix_l] - x, E_p[iy_l] - y)

Because softmax normalizes, only Z = sum_l exp, Sy = sum_l exp*iy, Sx = sum_l
exp*ix are needed per q (flash-attention style, no [L,L] materialization, and
exp(10*s - 30) needs no running max since |10*s| <= 90 and using a fixed shift
keeps everything in fp32 range).

Sharding: 8 cores = 4 batches x 2 query-row halves. Each core holds the full
K-side image (softmax runs over all 4096 l) and 2048 queries.

Performance notes (hw-measured):
 - The PE HAM clock gate keeps matmuls at 1.2 GHz until ~3.4us of sustained
   activity, then 2.4 GHz (N=512 matmul: 427 -> 216 ns). Dependency-free
   warmup matmuls gated on the padded images keep the PE warm through the
   patch-build DMA window so the main loop runs warm from its first tile.
 - exp on ScalarE costs (N+352)/1.2 ns; batching it over two PSUM banks
   ([128,1024]) amortizes the 352-cycle launch overhead.
 - The l2 normalize runs 128-partition-wide (4 pixel chunks stacked) so the
   DVE/ACT/recip work is 4x denser than a [32, n] layout allows.
"""

import sys

import numpy as np

sys.path.insert(0, "/opt/trn_rl_repo")

import concourse.bass as bass  # noqa: E402
import concourse.mybir as mybir  # noqa: E402
import concourse.tile as tile  # noqa: E402
from concourse import bacc, bass_utils  # noqa: E402

F32 = mybir.dt.float32
F32R = mybir.dt.float32r
F16 = mybir.dt.float16
BF16 = mybir.dt.bfloat16

B, C, H, W = 4, 32, 64, 64
L = H * W              # 4096 match locations
NQ = L // 2            # queries per core
QROWS = H // 2         # query rows per core
N_CORES = 8
SCALE = 10.0
SHIFT = -30.0          # exp(10*s - 30): |10*s|<=90 so no overflow, and a row's
                       # max 10*s is never < -60 so Z stays far above underflow
EPS = 1e-12
TAPS = [(dy, dx) for dy in range(3) for dx in range(3)]

_NC_CACHE = {}
_LAST_RES = None


def _build_nc():
    nc = bacc.Bacc(None, target_bir_lowering=False)

    f1h = nc.dram_tensor("f1h", [C, QROWS + 2, W], F32, kind="ExternalInput")
    f2 = nc.dram_tensor("f2", [C, H, W], F32, kind="ExternalInput")
    w3 = nc.dram_tensor("w3", [128, 96], F32, kind="ExternalInput")
    yq = nc.dram_tensor("yq", [128, 16], F32, kind="ExternalInput")
    xq = nc.dram_tensor("xq", [128, 16], F32, kind="ExternalInput")
    o4b = nc.dram_tensor("o4b", [4, 128], F32, kind="ExternalInput")
    outp = nc.dram_tensor("outp", [2, NQ], F32, kind="ExternalOutput")

    R1 = QROWS + 2         # 34 rows in the f1 halo slab
    # f1 row chunks for the 128-wide normalize stacking (9+9+8+8 rows)
    r1chunks = [(0, 9), (9, 18), (18, 26), (26, 34)]

    with tile.TileContext(nc) as tc:
        with tc.tile_pool(name="big", bufs=1) as big, \
             tc.tile_pool(name="work", bufs=1) as work, \
             tc.tile_pool(name="small", bufs=1) as small, \
             tc.tile_pool(name="pp", bufs=4) as pp, \
             tc.tile_pool(name="epi", bufs=2) as epi, \
             tc.tile_pool(name="sps", bufs=3, space="PSUM") as sps, \
             tc.tile_pool(name="stps", bufs=2, space="PSUM") as stps:

            # ---- load inputs; 4 partition-stacked chunks per image ----
            raw2x = big.tile([128, 1024], F32, tag="raw2x")
            for a in range(4):
                eng = nc.sync if a % 2 == 0 else nc.scalar
                eng.dma_start(
                    out=raw2x[32 * a:32 * a + 32, :],
                    in_=f2[:, 16 * a:16 * a + 16, :].rearrange(
                        "c h w -> c (h w)"))
            raw1x = big.tile([128, 576], F32, tag="raw1x")
            nc.vector.memset(raw1x, 0.0)
            for a, (r0, r1) in enumerate(r1chunks):
                eng = nc.scalar if a % 2 == 0 else nc.sync
                eng.dma_start(
                    out=raw1x[32 * a:32 * a + 32, 0:(r1 - r0) * W],
                    in_=f1h[:, r0:r1, :].rearrange("c h w -> c (h w)"))
            w3f = small.tile([128, 96], F32, tag="w3f")
            nc.sync.dma_start(out=w3f, in_=w3[:, :])
            w3r = small.tile([128, 96], BF16, tag="w3r")
            nc.vector.tensor_copy(w3r, w3f)
            # stats weights padded to 128 columns per tile (125 zeros) so
            # the PE weight path never leaves FWL mode (a [128,3] stationary
            # disables fast-weight-load and costs ~100ns on each of the two
            # mode switches per block)
            w3p = small.tile([128, 32 * 128], BF16, tag="w3p")
            nc.gpsimd.memset(w3p, 0.0)
            nc.scalar.dma_start(
                out=w3p.rearrange("p (t c) -> p t c", c=128)[:, :, 0:3],
                in_=w3r.rearrange("p (t c) -> p t c", c=3))
            # stats weights padded to 128 columns per tile (125 zeros) so
            # the PE weight path never leaves FWL mode (a [128,3] stationary
            # disables fast-weight-load and costs ~100ns on each of the two
            # mode switches per block)
            w3p = small.tile([128, 32 * 128], BF16, tag="w3p")
            nc.gpsimd.memset(w3p, 0.0)
            nc.scalar.dma_start(
                out=w3p.rearrange("p (t c) -> p t c", c=128)[:, :, 0:3],
                in_=w3r.rearrange("p (t c) -> p t c", c=3))
            xqs = small.tile([128, 16], F32, tag="xqs")
            nc.sync.dma_start(out=xqs, in_=xq[:, :])
            yqs = small.tile([128, 16], F32, tag="yqs")
            nc.sync.dma_start(out=yqs, in_=yq[:, :])

            # block-diagonal ones for per-chunk partition reductions and
            # broadcasts: ones4[32a+c, a'] = (a == a'); ones4b = transpose
            ones4f = small.tile([128, 4], F32, tag="ones4f")
            nc.vector.memset(ones4f, 0.0)
            for a in range(4):
                nc.vector.memset(ones4f[32 * a:32 * a + 32, a:a + 1], 1.0)
            ones4 = small.tile([128, 4], F32R, tag="ones4")
            nc.vector.tensor_copy(ones4, ones4f)
            ones4bf = small.tile([4, 128], F32, tag="ones4bf")
            nc.sync.dma_start(out=ones4bf, in_=o4b[:, :])
            ones4b = small.tile([4, 128], F32R, tag="ones4b")
            nc.vector.tensor_copy(ones4b, ones4bf)
            shiftc = small.tile([128, 1], F32, tag="shiftc")
            nc.vector.memset(shiftc, SHIFT)
            eps2c = small.tile([4, 1], F32, tag="eps2c")
            nc.vector.memset(eps2c, EPS * EPS)

            # ---- l2 normalization over C, 128-wide (4 pixel chunks stacked
            # on partitions). Per-pixel 1/sqrt(sum_c x^2) via block-ones
            # matmul -> sqrt -> 128-wide reciprocal -> block-ones broadcast
            # matmul -> scale-multiply, all on [128, n] tiles ----
            def normalize(rawx, ncols, img):
                sq = work.tile([128, 1024], F32R, tag="sq", name="sq", bufs=2)
                nc.vector.tensor_mul(sq[:, :ncols], rawx, rawx)
                nr = work.tile([4, 1024], F32, tag=f"nr{img}",
                               name=f"nr{img}")
                for j in range((ncols + 511) // 512):
                    n = min(512, ncols - 512 * j)
                    ss = sps.tile([128, 1024], F32, tag="s", name="ssp")
                    nc.tensor.matmul(ss[0:4, :n], ones4,
                                     sq[:, 512 * j:512 * j + n],
                                     start=True, stop=True)
                    nc.scalar.activation(nr[:, 512 * j:512 * j + n],
                                         ss[0:4, :n],
                                         mybir.ActivationFunctionType.Sqrt,
                                         bias=eps2c)
                # 1/norm via the custom-DVE Newton-Raphson reciprocal (~18
                # correct bits, layout-free) -- no transpose bounce needed
                rrf = work.tile([4, 1024], F32, tag=f"rrf{img}",
                                name=f"rrf{img}")
                nc.vector.reciprocal_approx_fast(rrf[:, :ncols],
                                                 nr[:, :ncols])
                rr = work.tile([4, 1024], F32R, tag=f"rr{img}",
                               name=f"rr{img}")
                with nc.allow_low_precision(reason="f32r 1/norm broadcast"):
                    nc.vector.tensor_copy(rr[:, :ncols], rrf[:, :ncols])
                pr = work.tile([128, 1024], F16, tag=f"pr{img}",
                               name=f"pr{img}")
                for j in range((ncols + 511) // 512):
                    n = min(512, ncols - 512 * j)
                    rb = stps.tile([128, 512], F32, tag="stats", name="rb")
                    nc.tensor.matmul(rb[:, :n], ones4b,
                                     rr[:, 512 * j:512 * j + n],
                                     start=True, stop=True)
                    nc.vector.tensor_mul(pr[:, 512 * j:512 * j + n],
                                         rawx[:, 512 * j:512 * j + n],
                                         rb[:, :n])
                return pr

            # ---- d-major patch tensors: 3 groups (dy) of 3 taps (dx);
            # tap (g, j) holds the normalized image shifted by (g-1, j-1),
            # DMA'd straight from the 128-wide pr tensors (spread SBUF read
            # ports). Tensors are split into row-chunk tiles so the main
            # loop starts as soon as the first chunks land; later chunks
            # stream in under the first blocks' compute ----
            kpc = [[big.tile([96, 16, W], F16, tag=f"kp{g}_{a}",
                             name=f"kp{g}_{a}") for a in range(4)]
                   for g in range(3)]
            qpc = [[big.tile([96, 8, W], F16, tag=f"qp{g}_{t}",
                             name=f"qp{g}_{t}") for t in range(4)]
                   for g in range(3)]
            dma_engs = [nc.sync, nc.scalar]

            # border zeros: g=0 top image row, g=2 bottom, and the x edges
            # (tap j=0 on partitions 0:32, j=2 on 64:96)
            nc.gpsimd.memset(kpc[0][0][:, 0:1, :], 0.0)
            nc.gpsimd.memset(kpc[2][3][:, 15:16, :], 0.0)
            for g in range(3):
                for a in range(4):
                    nc.gpsimd.memset(kpc[g][a][0:32, :, 0:1], 0.0)
                    nc.gpsimd.memset(kpc[g][a][64:96, :, W - 1:W], 0.0)
                for t in range(4):
                    nc.gpsimd.memset(qpc[g][t][0:32, :, 0:1], 0.0)
                    nc.gpsimd.memset(qpc[g][t][64:96, :, W - 1:W], 0.0)

            pr2 = normalize(raw2x, 1024, img=2)
            di = [0]

            def kp_chunk(a, engs):
                for g in range(3):
                    for j in range(3):
                        x0 = max(0, 1 - j)
                        x1 = min(W, W + 1 - j)
                        y0 = max(16 * a, max(0, 1 - g))
                        y1 = min(16 * a + 16, min(H, H + 1 - g))
                        y = y0
                        while y < y1:
                            sy = y + g - 1                 # src image row
                            ca = sy // 16
                            n = min(y1 + g - 1, 16 * ca + 16) - sy
                            src = pr2[32 * ca:32 * ca + 32, :].rearrange(
                                "c (h w) -> c h w", w=W)
                            engs[di[0] % len(engs)].dma_start(
                                out=kpc[g][a][32 * j:32 * j + 32,
                                              y - 16 * a:y - 16 * a + n,
                                              x0:x1],
                                in_=src[:, sy - 16 * ca:sy - 16 * ca + n,
                                        x0 + j - 1:x1 + j - 1])
                            di[0] += 1
                            y += n

            # f1 comes as a halo slab (row 0 = image row -1, zero-filled on
            # host at global edges), so dy shifts never leave the slab
            b1 = [0, 9, 18, 26, 34]

            def qp_slab(t, engs, pr1):
                for g in range(3):
                    for j in range(3):
                        x0 = max(0, 1 - j)
                        x1 = min(W, W + 1 - j)
                        y = 8 * t
                        while y < 8 * t + 8:
                            sy = y + g                     # src slab row
                            ca = max(c for c in range(4) if b1[c] <= sy)
                            n = min(8 * t + 8 + g, b1[ca + 1]) - sy
                            src = pr1[32 * ca:32 * ca + 32, :].rearrange(
                                "c (h w) -> c h w", w=W)
                            engs[di[0] % len(engs)].dma_start(
                                out=qpc[g][t][32 * j:32 * j + 32,
                                              y - 8 * t:y - 8 * t + n,
                                              x0:x1],
                                in_=src[:, sy - b1[ca]:sy - b1[ca] + n,
                                        x0 + j - 1:x1 + j - 1])
                            di[0] += 1
                            y += n

            # loop-start critical set on all 3 queues; the rest streams on
            # sync/gpsimd under the loop (scalar stays free for exp)
            kp_chunk(0, dma_engs)
            pr1 = normalize(raw1x, 576, img=1)
            qp_slab(0, dma_engs, pr1)
            kp_chunk(1, [nc.scalar])   # fits in scalar's pre-loop idle time
            for a in (2, 3):
                kp_chunk(a, dma_engs[:1])
            for t in (1, 2, 3):
                qp_slab(t, dma_engs[:1], pr1)

            # ---- PE warmup: HAM lifts the PE clock gate (1.2 -> 2.4 GHz)
            # only after ~3.4us of sustained activity. These matmuls are
            # data-gated on preamble products, so the scheduler runs them
            # during the normalize/patch-DMA window right before the main
            # loop -- entering it warm ----
            for i in range(6):
                wps = sps.tile([128, 1024], F32, tag="s", name="wps")
                nc.tensor.matmul(wps[:, 0:512], pr2[:, 0:128],
                                 pr2[:, 0:512], start=True, stop=True)
            for i in range(4):
                wps = sps.tile([128, 1024], F32, tag="s", name="wps")
                nc.tensor.matmul(wps[:, 0:512], pr1[:, 0:128],
                                 pr1[:, 0:512], start=True, stop=True)
            for g in range(3):
                wps = sps.tile([128, 1024], F32, tag="s", name="wps")
                nc.tensor.matmul(wps[:, 0:512], kpc[g][0][:, 0:2, :],
                                 kpc[g][0][:, 2:10, :],
                                 start=True, stop=True)
            for g in range(3):
                wps = sps.tile([128, 1024], F32, tag="s", name="wps")
                nc.tensor.matmul(wps[:, 0:512], qpc[g][0][:, 0:2, :],
                                 qpc[g][0][:, 0:8, :],
                                 start=True, stop=True)

            # ---- main loop: scores -> exp -> stats, flash-attention style.
            # Two 128-l tiles per block share one [128,1024] PSUM pair and
            # one batched exp; the stats matmuls run one block behind so the
            # in-order PE never waits on the exp ----
            n_bt = (L // 128) // 2
            n_qt = NQ // 512
            for qt in range(n_qt):
                stats = stps.tile([128, 512], F32, tag="stats")
                pend = []
                for bt in range(n_bt):
                    s2 = sps.tile([128, 1024], F32, tag="s")
                    for half in range(2):
                        lt = 2 * bt + half
                        for g in range(3):
                            nc.tensor.matmul(
                                s2[:, 512 * half:512 * half + 512],
                                kpc[g][lt // 8][:, 2 * (lt % 8):
                                                2 * (lt % 8) + 2, :],
                                qpc[g][qt][:, :, :],
                                start=(g == 0), stop=(g == 2),
                            )
                    if len(pend) == 2:
                        pbt, pp2 = pend.pop(0)
                        for half in range(2):
                            plt = 2 * pbt + half
                            nc.tensor.matmul(
                                stats, w3p[:, 128 * plt:128 * plt + 128],
                                pp2[:, 512 * half:512 * half + 512],
                                start=(plt == 0), stop=False)
                    p2 = pp.tile([128, 1024], BF16, tag="p")
                    nc.scalar.activation(p2, s2,
                                         mybir.ActivationFunctionType.Exp,
                                         bias=shiftc, scale=SCALE)
                    pend.append((bt, p2))
                for pbt, pp2 in pend:
                    for half in range(2):
                        plt = 2 * pbt + half
                        nc.tensor.matmul(stats,
                                         w3p[:, 128 * plt:128 * plt + 128],
                                         pp2[:, 512 * half:512 * half + 512],
                                         start=False,
                                         stop=(plt == L // 128 - 1))

                # flow = S/Z - coord, all 128-wide: stats rows land as
                # [128, 4] blocks (q = 128*c + p) so the reciprocal and
                # elementwise tail are dense
                st3 = epi.tile([3, 512], F32, tag="st3")
                nc.scalar.copy(st3, stats[0:3, :])
                tz = epi.tile([128, 4], F32, tag="tz")
                ty = epi.tile([128, 4], F32, tag="ty")
                tx = epi.tile([128, 4], F32, tag="tx")
                for r, t in enumerate((tz, ty, tx)):
                    dma_engs[r % 2].dma_start(
                        out=t, in_=st3[r:r + 1, :].rearrange(
                            "a (p c) -> a p c", p=128))
                rz = epi.tile([128, 4], F32, tag="rz")
                with nc.allow_low_precision(reason="f32 recip of Z"):
                    nc.vector.reciprocal(rz, tz)
                fw = epi.tile([128, 4], F32, tag="fw")
                nc.vector.tensor_mul(fw, tx, rz)
                nc.vector.tensor_sub(fw, fw, xqs[:, 4 * qt:4 * qt + 4])
                fh = epi.tile([128, 4], F32, tag="fh")
                nc.vector.tensor_mul(fh, ty, rz)
                nc.vector.tensor_sub(fh, fh, yqs[:, 4 * qt:4 * qt + 4])
                nc.sync.dma_start(
                    out=outp[0:1, 512 * qt:512 * qt + 512].rearrange(
                        "a (p c) -> a p c", p=128), in_=fw)
                nc.scalar.dma_start(
                    out=outp[1:2, 512 * qt:512 * qt + 512].rearrange(
                        "a (p c) -> a p c", p=128), in_=fh)

    nc.finalize()
    return nc


def _host_consts():
    p = np.arange(128)
    w3 = np.zeros((128, 96), np.float32)
    for t in range(32):
        w3[:, 3 * t] = 1.0
        w3[:, 3 * t + 1] = 2 * t + p // 64   # global iy of l = 128*lt + p
        w3[:, 3 * t + 2] = p % 64            # global ix
    # query coords in the epilogue's [128, 4] layout: q = 512*qt + 4*p + c
    j = np.arange(16)[None, :]
    q = 512 * (j // 4) + 4 * p[:, None] + (j % 4)    # [128, 16] global q
    xq = (q % W).astype(np.float32)
    ly = (q // W).astype(np.float32)
    o4b = np.zeros((4, 128), np.float32)
    for a in range(4):
        o4b[a, 32 * a:32 * a + 32] = 1.0
    return w3, xq, ly, o4b


def kernel(feature1, feature2):
    feature1 = np.ascontiguousarray(feature1, np.float32)
    feature2 = np.ascontiguousarray(feature2, np.float32)
    w3, xq, ly, o4b = _host_consts()

    f1p = np.zeros((B, C, H + 2, W), np.float32)
    f1p[:, :, 1:H + 1, :] = feature1

    in_maps = []
    for core in range(N_CORES):
        b, h = divmod(core, 2)
        in_maps.append({
            "f1h": np.ascontiguousarray(f1p[b, :, h * QROWS:h * QROWS + QROWS + 2, :]),
            "f2": np.ascontiguousarray(feature2[b]),
            "w3": w3,
            "yq": (ly + h * QROWS).astype(np.float32),
            "xq": xq,
            "o4b": o4b,
        })

    if "nc" not in _NC_CACHE:
        _NC_CACHE["nc"] = _build_nc()
    res = bass_utils.run_bass_kernel_spmd(
        _NC_CACHE["nc"], in_maps, core_ids=list(range(N_CORES)))
    global _LAST_RES
    _LAST_RES = res

    out = np.zeros((B, 2, H, W), np.float32)
    for core in range(N_CORES):
        b, h = divmod(core, 2)
        out[b, :, h * QROWS:(h + 1) * QROWS, :] = (
            res.results[core]["outp"].reshape(2, QROWS, W))
    return out


# revision 25
# speedup vs baseline: 1.0104x; 1.0104x over previous
"""Trainium2 Bass kernel for the correlation-softargmax flow module.

Math (per batch b, query pixel q=(y,x)):
  c1 = l2norm_C(feature1), warp = l2norm_C(feature2)
  s[l,q] = <3x3 patch of warp at l, 3x3 patch of c1 at q>    (D = 32*9 = 288)
  p = softmax_l(10*s);  flow = (E_p[ix_l] - x, E_p[iy_l] - y)

Because softmax normalizes, only Z = sum_l exp, Sy = sum_l exp*iy, Sx = sum_l
exp*ix are needed per q (flash-attention style, no [L,L] materialization, and
exp(10*s - 30) needs no running max since |10*s| <= 90 and using a fixed shift
keeps everything in fp32 range).

Sharding: 8 cores = 4 batches x 2 query-row halves. Each core holds the full
K-side image (softmax runs over all 4096 l) and 2048 queries.

Performance notes (hw-measured):
 - The PE HAM clock gate keeps matmuls at 1.2 GHz until ~3.4us of sustained
   activity, then 2.4 GHz (N=512 matmul: 427 -> 216 ns). Dependency-free
   warmup matmuls gated on the padded images keep the PE warm through the
   patch-build DMA window so the main loop runs warm from its first tile.
 - exp on ScalarE costs (N+352)/1.2 ns; batching it over two PSUM banks
   ([128,1024]) amortizes the 352-cycle launch overhead.
 - The l2 normalize runs 128-partition-wide (4 pixel chunks stacked) so the
   DVE/ACT/recip work is 4x denser than a [32, n] layout allows.
"""

import sys

import numpy as np

sys.path.insert(0, "/opt/trn_rl_repo")

import concourse.bass as bass  # noqa: E402
import concourse.mybir as mybir  # noqa: E402
import concourse.tile as tile  # noqa: E402
from concourse import bacc, bass_utils  # noqa: E402

F32 = mybir.dt.float32
F32R = mybir.dt.float32r
F16 = mybir.dt.float16
BF16 = mybir.dt.bfloat16

B, C, H, W = 4, 32, 64, 64
L = H * W              # 4096 match locations
NQ = L // 2            # queries per core
QROWS = H // 2         # query rows per core
N_CORES = 8
SCALE = 10.0
SHIFT = -30.0          # exp(10*s - 30): |10*s|<=90 so no overflow, and a row's
                       # max 10*s is never < -60 so Z stays far above underflow
EPS = 1e-12
TAPS = [(dy, dx) for dy in range(3) for dx in range(3)]

_NC_CACHE = {}
_LAST_RES = None


def _build_nc():
    nc = bacc.Bacc(None, target_bir_lowering=False)

    f1h = nc.dram_tensor("f1h", [C, QROWS + 2, W], F32, kind="ExternalInput")
    f2 = nc.dram_tensor("f2", [C, H, W], F32, kind="ExternalInput")
    w3 = nc.dram_tensor("w3", [128, 96], F32, kind="ExternalInput")
    yq = nc.dram_tensor("yq", [128, 16], F32, kind="ExternalInput")
    xq = nc.dram_tensor("xq", [128, 16], F32, kind="ExternalInput")
    o4b = nc.dram_tensor("o4b", [4, 128], F32, kind="ExternalInput")
    outp = nc.dram_tensor("outp", [2, NQ], F32, kind="ExternalOutput")

    R1 = QROWS + 2         # 34 rows in the f1 halo slab
    # f1 row chunks for the 128-wide normalize stacking (9+9+8+8 rows)
    r1chunks = [(0, 9), (9, 18), (18, 26), (26, 34)]

    with tile.TileContext(nc) as tc:
        with tc.tile_pool(name="big", bufs=1) as big, \
             tc.tile_pool(name="work", bufs=1) as work, \
             tc.tile_pool(name="small", bufs=1) as small, \
             tc.tile_pool(name="pp", bufs=4) as pp, \
             tc.tile_pool(name="epi", bufs=2) as epi, \
             tc.tile_pool(name="sps", bufs=3, space="PSUM") as sps, \
             tc.tile_pool(name="stps", bufs=2, space="PSUM") as stps:

            # ---- load inputs; 4 partition-stacked chunks per image ----
            raw2x = big.tile([128, 1024], F32, tag="raw2x")
            for a in range(4):
                eng = nc.sync if a % 2 == 0 else nc.scalar
                eng.dma_start(
                    out=raw2x[32 * a:32 * a + 32, :],
                    in_=f2[:, 16 * a:16 * a + 16, :].rearrange(
                        "c h w -> c (h w)"))
            raw1x = big.tile([128, 576], F32, tag="raw1x")
            nc.vector.memset(raw1x, 0.0)
            for a, (r0, r1) in enumerate(r1chunks):
                eng = nc.scalar if a % 2 == 0 else nc.sync
                eng.dma_start(
                    out=raw1x[32 * a:32 * a + 32, 0:(r1 - r0) * W],
                    in_=f1h[:, r0:r1, :].rearrange("c h w -> c (h w)"))
            w3f = small.tile([128, 96], F32, tag="w3f")
            nc.sync.dma_start(out=w3f, in_=w3[:, :])
            w3r = small.tile([128, 96], BF16, tag="w3r")
            nc.vector.tensor_copy(w3r, w3f)
            # stats weights padded to 128 columns per tile (125 zeros) so
            # the PE weight path never leaves FWL mode (a [128,3] stationary
            # disables fast-weight-load and costs ~100ns on each of the two
            # mode switches per block)
            w3p = small.tile([128, 32 * 128], BF16, tag="w3p")
            nc.gpsimd.memset(w3p, 0.0)
            nc.scalar.dma_start(
                out=w3p.rearrange("p (t c) -> p t c", c=128)[:, :, 0:3],
                in_=w3r.rearrange("p (t c) -> p t c", c=3))
            # stats weights padded to 128 columns per tile (125 zeros) so
            # the PE weight path never leaves FWL mode (a [128,3] stationary
            # disables fast-weight-load and costs ~100ns on each of the two
            # mode switches per block)
            w3p = small.tile([128, 32 * 128], BF16, tag="w3p")
            nc.gpsimd.memset(w3p, 0.0)
            nc.scalar.dma_start(
                out=w3p.rearrange("p (t c) -> p t c", c=128)[:, :, 0:3],
                in_=w3r.rearrange("p (t c) -> p t c", c=3))
            xqs = small.tile([128, 16], F32, tag="xqs")
            nc.sync.dma_start(out=xqs, in_=xq[:, :])
            yqs = small.tile([128, 16], F32, tag="yqs")
            nc.sync.dma_start(out=yqs, in_=yq[:, :])

            # block-diagonal ones for per-chunk partition reductions and
            # broadcasts: ones4[32a+c, a'] = (a == a'); ones4b = transpose
            ones4f = small.tile([128, 4], F32, tag="ones4f")
            nc.vector.memset(ones4f, 0.0)
            for a in range(4):
                nc.vector.memset(ones4f[32 * a:32 * a + 32, a:a + 1], 1.0)
            ones4 = small.tile([128, 4], F32R, tag="ones4")
            nc.vector.tensor_copy(ones4, ones4f)
            ones4bf = small.tile([4, 128], F32, tag="ones4bf")
            nc.sync.dma_start(out=ones4bf, in_=o4b[:, :])
            ones4b = small.tile([4, 128], F32R, tag="ones4b")
            nc.vector.tensor_copy(ones4b, ones4bf)
            shiftc = small.tile([128, 1], F32, tag="shiftc")
            nc.vector.memset(shiftc, SHIFT)
            eps2c = small.tile([4, 1], F32, tag="eps2c")
            nc.vector.memset(eps2c, EPS * EPS)

            # ---- l2 normalization over C, 128-wide (4 pixel chunks stacked
            # on partitions). Per-pixel 1/sqrt(sum_c x^2) via block-ones
            # matmul -> sqrt -> 128-wide reciprocal -> block-ones broadcast
            # matmul -> scale-multiply, all on [128, n] tiles ----
            def normalize(rawx, ncols, img):
                sq = work.tile([128, 1024], F32R, tag="sq", name="sq", bufs=2)
                nc.vector.tensor_mul(sq[:, :ncols], rawx, rawx)
                nr = work.tile([4, 1024], F32, tag=f"nr{img}",
                               name=f"nr{img}")
                for j in range((ncols + 511) // 512):
                    n = min(512, ncols - 512 * j)
                    ss = sps.tile([128, 1024], F32, tag="s", name="ssp")
                    nc.tensor.matmul(ss[0:4, :n], ones4,
                                     sq[:, 512 * j:512 * j + n],
                                     start=True, stop=True)
                    nc.scalar.activation(nr[:, 512 * j:512 * j + n],
                                         ss[0:4, :n],
                                         mybir.ActivationFunctionType.Sqrt,
                                         bias=eps2c)
                # 1/norm via the custom-DVE Newton-Raphson reciprocal (~18
                # correct bits, layout-free) -- no transpose bounce needed
                rrf = work.tile([4, 1024], F32, tag=f"rrf{img}",
                                name=f"rrf{img}")
                nc.vector.reciprocal_approx_fast(rrf[:, :ncols],
                                                 nr[:, :ncols])
                rr = work.tile([4, 1024], F32R, tag=f"rr{img}",
                               name=f"rr{img}")
                with nc.allow_low_precision(reason="f32r 1/norm broadcast"):
                    nc.vector.tensor_copy(rr[:, :ncols], rrf[:, :ncols])
                pr = work.tile([128, 1024], F16, tag=f"pr{img}",
                               name=f"pr{img}")
                for j in range((ncols + 511) // 512):
                    n = min(512, ncols - 512 * j)
                    rb = stps.tile([128, 512], F32, tag="stats", name="rb")
                    nc.tensor.matmul(rb[:, :n], ones4b,
                                     rr[:, 512 * j:512 * j + n],
                                     start=True, stop=True)
                    nc.vector.tensor_mul(pr[:, 512 * j:512 * j + n],
                                         rawx[:, 512 * j:512 * j + n],
                                         rb[:, :n])
                return pr

            # ---- d-major patch tensors: 3 groups (dy) of 3 taps (dx);
            # tap (g, j) holds the normalized image shifted by (g-1, j-1),
            # DMA'd straight from the 128-wide pr tensors (spread SBUF read
            # ports). Tensors are split into row-chunk tiles so the main
            # loop starts as soon as the first chunks land; later chunks
            # stream in under the first blocks' compute ----
            kpc = [[big.tile([96, 16, W], F16, tag=f"kp{g}_{a}",
                             name=f"kp{g}_{a}") for a in range(4)]
                   for g in range(3)]
            qpc = [[big.tile([96, 8, W], F16, tag=f"qp{g}_{t}",
                             name=f"qp{g}_{t}") for t in range(4)]
                   for g in range(3)]
            dma_engs = [nc.sync, nc.scalar]

            # border zeros: g=0 top image row, g=2 bottom, and the x edges
            # (tap j=0 on partitions 0:32, j=2 on 64:96)
            nc.gpsimd.memset(kpc[0][0][:, 0:1, :], 0.0)
            nc.gpsimd.memset(kpc[2][3][:, 15:16, :], 0.0)
            for g in range(3):
                for a in range(4):
                    nc.gpsimd.memset(kpc[g][a][0:32, :, 0:1], 0.0)
                    nc.gpsimd.memset(kpc[g][a][64:96, :, W - 1:W], 0.0)
                for t in range(4):
                    nc.gpsimd.memset(qpc[g][t][0:32, :, 0:1], 0.0)
                    nc.gpsimd.memset(qpc[g][t][64:96, :, W - 1:W], 0.0)

            pr2 = normalize(raw2x, 1024, img=2)
            di = [0]

            def kp_chunk(a, engs):
                for g in range(3):
                    for j in range(3):
                        x0 = max(0, 1 - j)
                        x1 = min(W, W + 1 - j)
                        y0 = max(16 * a, max(0, 1 - g))
                        y1 = min(16 * a + 16, min(H, H + 1 - g))
                        y = y0
                        while y < y1:
                            sy = y + g - 1                 # src image row
                            ca = sy // 16
                            n = min(y1 + g - 1, 16 * ca + 16) - sy
                            src = pr2[32 * ca:32 * ca + 32, :].rearrange(
                                "c (h w) -> c h w", w=W)
                            engs[di[0] % len(engs)].dma_start(
                                out=kpc[g][a][32 * j:32 * j + 32,
                                              y - 16 * a:y - 16 * a + n,
                                              x0:x1],
                                in_=src[:, sy - 16 * ca:sy - 16 * ca + n,
                                        x0 + j - 1:x1 + j - 1])
                            di[0] += 1
                            y += n

            # f1 comes as a halo slab (row 0 = image row -1, zero-filled on
            # host at global edges), so dy shifts never leave the slab
            b1 = [0, 9, 18, 26, 34]

            def qp_slab(t, engs, pr1):
                for g in range(3):
                    for j in range(3):
                        x0 = max(0, 1 - j)
                        x1 = min(W, W + 1 - j)
                        y = 8 * t
                        while y < 8 * t + 8:
                            sy = y + g                     # src slab row
                            ca = max(c for c in range(4) if b1[c] <= sy)
                            n = min(8 * t + 8 + g, b1[ca + 1]) - sy
                            src = pr1[32 * ca:32 * ca + 32, :].rearrange(
                                "c (h w) -> c h w", w=W)
                            engs[di[0] % len(engs)].dma_start(
                                out=qpc[g][t][32 * j:32 * j + 32,
                                              y - 8 * t:y - 8 * t + n,
                                              x0:x1],
                                in_=src[:, sy - b1[ca]:sy - b1[ca] + n,
                                        x0 + j - 1:x1 + j - 1])
                            di[0] += 1
                            y += n

            # loop-start critical set on all 3 queues; the rest streams on
            # sync/gpsimd under the loop (scalar stays free for exp)
            kp_chunk(0, dma_engs)
            pr1 = normalize(raw1x, 576, img=1)
            qp_slab(0, dma_engs, pr1)
            for a in (1, 2, 3):
                kp_chunk(a, dma_engs[:1])
            for t in (1, 2, 3):
                qp_slab(t, dma_engs[:1], pr1)

            # ---- PE warmup: HAM lifts the PE clock gate (1.2 -> 2.4 GHz)
            # only after ~3.4us of sustained activity. These matmuls are
            # data-gated on preamble products, so the scheduler runs them
            # during the normalize/patch-DMA window right before the main
            # loop -- entering it warm ----
            for i in range(6):
                wps = sps.tile([128, 1024], F32, tag="s", name="wps")
                nc.tensor.matmul(wps[:, 0:512], pr2[:, 0:128],
                                 pr2[:, 0:512], start=True, stop=True)
            for i in range(4):
                wps = sps.tile([128, 1024], F32, tag="s", name="wps")
                nc.tensor.matmul(wps[:, 0:512], pr1[:, 0:128],
                                 pr1[:, 0:512], start=True, stop=True)
            for g in range(3):
                wps = sps.tile([128, 1024], F32, tag="s", name="wps")
                nc.tensor.matmul(wps[:, 0:512], kpc[g][0][:, 0:2, :],
                                 kpc[g][0][:, 2:10, :],
                                 start=True, stop=True)
            for g in range(3):
                wps = sps.tile([128, 1024], F32, tag="s", name="wps")
                nc.tensor.matmul(wps[:, 0:512], qpc[g][0][:, 0:2, :],
                                 qpc[g][0][:, 0:8, :],
                                 start=True, stop=True)

            # ---- main loop: scores -> exp -> stats, flash-attention style.
            # Two 128-l tiles per block share one [128,1024] PSUM pair and
            # one batched exp; the stats matmuls run one block behind so the
            # in-order PE never waits on the exp ----
            n_bt = (L // 128) // 2
            n_qt = NQ // 512
            for qt in range(n_qt):
                stats = stps.tile([128, 512], F32, tag="stats")
                pend = []
                for bt in range(n_bt):
                    s2 = sps.tile([128, 1024], F32, tag="s")
                    for half in range(2):
                        lt = 2 * bt + half
                        for g in range(3):
                            nc.tensor.matmul(
                                s2[:, 512 * half:512 * half + 512],
                                kpc[g][lt // 8][:, 2 * (lt % 8):
                                                2 * (lt % 8) + 2, :],
                                qpc[g][qt][:, :, :],
                                start=(g == 0), stop=(g == 2),
                            )
                    if len(pend) == 2:
                        pbt, pp2 = pend.pop(0)
                        for half in range(2):
                            plt = 2 * pbt + half
                            nc.tensor.matmul(
                                stats, w3p[:, 128 * plt:128 * plt + 128],
                                pp2[:, 512 * half:512 * half + 512],
                                start=(plt == 0), stop=False)
                    p2 = pp.tile([128, 1024], BF16, tag="p")
                    nc.scalar.activation(p2, s2,
                                         mybir.ActivationFunctionType.Exp,
                                         bias=shiftc, scale=SCALE)
                    pend.append((bt, p2))
                for pbt, pp2 in pend:
                    for half in range(2):
                        plt = 2 * pbt + half
                        nc.tensor.matmul(stats,
                                         w3p[:, 128 * plt:128 * plt + 128],
                                         pp2[:, 512 * half:512 * half + 512],
                                         start=False,
                                         stop=(plt == L // 128 - 1))

                # flow = S/Z - coord, all 128-wide: stats rows land as
                # [128, 4] blocks (q = 128*c + p) so the reciprocal and
                # elementwise tail are dense
                st3 = epi.tile([3, 512], F32, tag="st3")
                nc.scalar.copy(st3, stats[0:3, :])
                tz = epi.tile([128, 4], F32, tag="tz")
                ty = epi.tile([128, 4], F32, tag="ty")
                tx = epi.tile([128, 4], F32, tag="tx")
                for r, t in enumerate((tz, ty, tx)):
                    dma_engs[r % 2].dma_start(
                        out=t, in_=st3[r:r + 1, :].rearrange(
                            "a (p c) -> a p c", p=128))
                rz = epi.tile([128, 4], F32, tag="rz")
                with nc.allow_low_precision(reason="f32 recip of Z"):
                    nc.vector.reciprocal(rz, tz)
                fw = epi.tile([128, 4], F32, tag="fw")
                nc.vector.tensor_mul(fw, tx, rz)
                nc.vector.tensor_sub(fw, fw, xqs[:, 4 * qt:4 * qt + 4])
                fh = epi.tile([128, 4], F32, tag="fh")
                nc.vector.tensor_mul(fh, ty, rz)
                nc.vector.tensor_sub(fh, fh, yqs[:, 4 * qt:4 * qt + 4])
                nc.sync.dma_start(
                    out=outp[0:1, 512 * qt:512 * qt + 512].rearrange(
                        "a (p c) -> a p c", p=128), in_=fw)
                nc.scalar.dma_start(
                    out=outp[1:2, 512 * qt:512 * qt + 512].rearrange(
                        "a (p c) -> a p c", p=128), in_=fh)

    nc.finalize()
    return nc


def _host_consts():
    p = np.arange(128)
    w3 = np.zeros((128, 96), np.float32)
    for t in range(32):
        w3[:, 3 * t] = 1.0
        w3[:, 3 * t + 1] = 2 * t + p // 64   # global iy of l = 128*lt + p
        w3[:, 3 * t + 2] = p % 64            # global ix
    # query coords in the epilogue's [128, 4] layout: q = 512*qt + 4*p + c
    j = np.arange(16)[None, :]
    q = 512 * (j // 4) + 4 * p[:, None] + (j % 4)    # [128, 16] global q
    xq = (q % W).astype(np.float32)
    ly = (q // W).astype(np.float32)
    o4b = np.zeros((4, 128), np.float32)
    for a in range(4):
        o4b[a, 32 * a:32 * a + 32] = 1.0
    return w3, xq, ly, o4b


def kernel(feature1, feature2):
    feature1 = np.ascontiguousarray(feature1, np.float32)
    feature2 = np.ascontiguousarray(feature2, np.float32)
    w3, xq, ly, o4b = _host_consts()

    f1p = np.zeros((B, C, H + 2, W), np.float32)
    f1p[:, :, 1:H + 1, :] = feature1

    in_maps = []
    for core in range(N_CORES):
        b, h = divmod(core, 2)
        in_maps.append({
            "f1h": np.ascontiguousarray(f1p[b, :, h * QROWS:h * QROWS + QROWS + 2, :]),
            "f2": np.ascontiguousarray(feature2[b]),
            "w3": w3,
            "yq": (ly + h * QROWS).astype(np.float32),
            "xq": xq,
            "o4b": o4b,
        })

    if "nc" not in _NC_CACHE:
        _NC_CACHE["nc"] = _build_nc()
    res = bass_utils.run_bass_kernel_spmd(
        _NC_CACHE["nc"], in_maps, core_ids=list(range(N_CORES)))
    global _LAST_RES
    _LAST_RES = res

    out = np.zeros((B, 2, H, W), np.float32)
    for core in range(N_CORES):
        b, h = divmod(core, 2)
        out[b, :, h * QROWS:(h + 1) * QROWS, :] = (
            res.results[core]["outp"].reshape(2, QROWS, W))
    return out
